# revision 4
# baseline (speedup 1.0000x reference)
"""Trainium2 Bass kernel v2 for the 3-layer PointGNN-style edge-scored GNN.

Host folds weights so that per layer:
    x' = X@Wx + bx ; B = X@Wb + bb ; A = X@Wa + ba
    h = relu(A[dst] + B[src]) ; s = sigmoid(h.w2 + b2)
    out[d] = sum_{e: dst=d} s_e * x'[src_e]       (+relu for layers 1,2)

v2 design (8-core SPMD, dst-partitioned, identical program per core; all
per-core variation comes through input data):
  - all matmuls in bf16 (PSUM accumulates fp32): 4x PE throughput vs fp32.
  - nodes permuted into nb bins of 128 slots (in-degree balanced); per-bin
    edge groups padded to uniform g_pad.
  - node phase (all bins, redundant per core) writes a DRAM row table per
    layer (bf16), 8 bins per batched dma_start:
      layers 0,1: rows [x' | B]            (2co = 128 cols)
      layer  2:   rows [X1 | 1 | pad | B]  (384 cols; X1 via identity-block
                  rhs so the transpose is free inside the same matmul)
  - layer 2 exploits linearity of the output head: out[d] =
    (sum_e s_e*[X1[src_e]|1]) @ [W';bx], so only the 64-dim X1 is gathered
    per edge and the 64->256 linear runs once per node at the end.
  - edge phase (own 30 bins): dma_gather rows by src (1024-idx chunks);
    host-built one-hot pairs [oh | ohT] (bf16) streamed from DRAM in
    16-tile chunks; PE does A-gather (ohT @ a_sb) + B-add (ident @ B) into
    PSUM and the dst scatter matmul; DVE does the fused relu*w2 row-reduce
    (score); ACT does sigmoid (batched over SIGK tiles) and the msg scale
    (layer 1's msg on Pool to balance engines).
  - layers 0,1 write xloc (bf16, feature-major) and AllGather it for the
    next layer's node phase.
"""

import sys

if "/opt/trn_rl_repo" not in sys.path:
    sys.path.insert(0, "/opt/trn_rl_repo")

import numpy as np

import concourse.bacc as bacc
import concourse.bass as bass  # noqa: F401
import concourse.mybir as mybir
import concourse.tile as tile
from concourse.bass_utils import run_bass_kernel_spmd

F32 = mybir.dt.float32
BF16 = mybir.dt.bfloat16
I16 = mybir.dt.int16
AF = mybir.ActivationFunctionType
ALU = mybir.AluOpType

NPBF16 = mybir.dt.np(BF16)

P = 128
NCORES = 8
GCH = 8         # tiles per gather chunk (1024 idx SWDGE packet cap)
OCH = 16        # tiles per one-hot stream chunk
SIGK = 4        # sigmoid batching (tiles per activation)
TWB = 8         # bins per batched table write
T2W = 384       # layer-2 table row width: [X1(64) | 1 | pad(63) | B(256)]
SIM_MODE = False  # replace collectives with local copies (CoreSim support)


class Cfg:
    def __init__(self, n_real, nbc, dims):
        self.n_real = n_real
        self.nbc = nbc
        self.nb = nbc * NCORES
        self.ng = self.nb * P
        self.dims = dims


CFG = Cfg(30000, 30, [(256, 64), (64, 64), (64, 256)])


# ---------------------------------------------------------------- host prep

def _balance_bins(weight, nb):
    """Assign nodes to nb bins of <=128 slots, balancing sum(weight)."""
    import heapq

    n = weight.shape[0]
    order = np.argsort(-weight, kind="stable")
    bin_of = np.empty(n, np.int32)
    slot_of = np.empty(n, np.int32)
    counts = np.zeros(nb, np.int32)
    heap = [(0, b) for b in range(nb)]
    heapq.heapify(heap)
    for i in order:
        spill = []
        while True:
            load, b = heapq.heappop(heap)
            if counts[b] < P:
                break
            spill.append((load, b))
        for s in spill:
            heapq.heappush(heap, s)
        bin_of[i] = b
        slot_of[i] = counts[b]
        counts[b] += 1
        heapq.heappush(heap, (load + int(weight[i]), b))
    return bin_of, slot_of


def _wrap16(flat_idx):
    n = flat_idx.shape[0]
    a = flat_idx.reshape(n // 16, 16).T.astype(np.int16)
    return np.tile(a, (8, 1))


def _host_prep(x, src, dst, cfg):
    n = cfg.n_real
    loops = np.arange(n, dtype=np.int64)
    src_all = np.concatenate([src, loops])
    dst_all = np.concatenate([dst, loops])

    indeg = np.bincount(dst_all, minlength=n).astype(np.int64)
    bin_of, slot_of = _balance_bins(indeg, cfg.nb)
    g_of = bin_of.astype(np.int64) * P + slot_of

    e_bin = bin_of[dst_all]
    order = np.argsort(e_bin, kind="stable")
    sb = e_bin[order]
    counts = np.bincount(e_bin, minlength=cfg.nb)
    g_pad = int(np.ceil(max(counts.max(), 1) / P) * P)
    starts = np.zeros(cfg.nb, np.int64)
    starts[1:] = np.cumsum(counts)[:-1]
    rank = np.arange(sb.shape[0]) - starts[sb]

    src_g = np.zeros((cfg.nb, g_pad), np.int64)            # pad edges -> row 0
    dst_slot = np.full((cfg.nb, g_pad), 255, np.int64)     # pad -> no match
    src_g[sb, rank] = g_of[src_all[order]]
    dst_slot[sb, rank] = slot_of[dst_all[order]]

    nt_e = g_pad // P
    per_core = []
    for c in range(NCORES):
        bins = slice(c * cfg.nbc, (c + 1) * cfg.nbc)
        sg = src_g[bins]
        srcw = np.concatenate([_wrap16(sg[t]) for t in range(cfg.nbc)], axis=1)

        # one-hot blob [128, n_gt*256] bf16: per tile cols [0:128]=oh
        # (oh[e,slot]), cols [128:256]=ohT (ohT[slot,e]).
        ds = dst_slot[bins].reshape(cfg.nbc * nt_e, P)      # [n_gt, 128 edges]
        n_gt = ds.shape[0]
        ohd = np.zeros((P, n_gt, 2 * P), NPBF16)
        gt_idx = np.repeat(np.arange(n_gt), P)
        e_idx = np.tile(np.arange(P), n_gt)
        s_idx = ds.reshape(-1)
        valid = s_idx < P
        ohd[e_idx[valid], gt_idx[valid], s_idx[valid]] = 1.0
        ohd[s_idx[valid], gt_idx[valid], P + e_idx[valid]] = 1.0
        per_core.append((srcw, ohd.reshape(P, n_gt * 2 * P)))

    c_in = cfg.dims[0][0]
    xa_full = np.zeros((c_in, cfg.ng), NPBF16)
    xa_full[:, g_of] = x.T.astype(NPBF16)
    return g_of, g_pad, per_core, xa_full


def _fuse_weights(ws, cfg):
    out = []
    for li, (ci, co) in enumerate(cfg.dims, start=1):
        wl = ws[f"w_lin{li}"].astype(np.float64)
        bl = ws[f"b_lin{li}"].astype(np.float64)
        ws1 = ws[f"w_s1_{li}"].astype(np.float64)
        bs1 = ws[f"b_s1_{li}"].astype(np.float64)
        ws2 = ws[f"w_s2_{li}"].astype(np.float64)
        bs2 = ws[f"b_s2_{li}"].astype(np.float64)
        wi, wj = ws1[:co], ws1[co:]
        wmat = np.zeros((ci + 1, 3 * co), np.float64)
        wmat[:ci, :co] = wl
        wmat[ci, :co] = bl
        wmat[:ci, co : 2 * co] = wl @ wj
        wmat[ci, co : 2 * co] = bl @ wj
        wmat[:ci, 2 * co :] = wl @ wi
        wmat[ci, 2 * co :] = bl @ wi + bs1
        d = dict(
            wmat=wmat.astype(NPBF16),
            w2=ws2[:, 0].astype(np.float32),
            b2=np.float32(bs2[0]),
        )
        if li == len(cfg.dims):
            # layer-2 node-table weights: rows [X1 | 1 | pad | B]
            w3n = np.zeros((ci, T2W), np.float64)
            w3n[:, 0:ci] = np.eye(ci)
            w3n[:, P : P + co] = wl @ wj
            w3nb = np.zeros((1, T2W), np.float64)
            w3nb[0, ci] = 1.0
            w3nb[0, P : P + co] = bl @ wj
            d["w3n"] = w3n.astype(NPBF16)
            d["w3nb"] = w3nb.astype(NPBF16)
        out.append(d)
    return out


# ---------------------------------------------------------------- program

def _build_program(cfg, g_pad):
    nbc, ng = cfg.nbc, cfg.ng
    nt_e = g_pad // P
    dims = cfg.dims
    nl = len(dims)
    c_in1 = dims[0][0]
    c_out_last = dims[-1][1]
    n_loc = nbc * P
    k1 = c_in1 // P                 # lhsT k-chunks for layer 1
    nb1 = max(1, nbc // 2)          # bins per L1 node super-chunk
    nch1 = (cfg.nb + nb1 - 1) // nb1
    n_gt = nbc * nt_e
    rw = [2 * dims[0][1], 2 * dims[1][1], T2W]   # table row widths

    # fp32 const blob [128, cf_cols]: w2 replicated rows + b2 columns
    c_w2 = []
    off = 0
    for l in range(nl):
        c_w2.append(off)
        off += dims[l][1]
    c_b2 = list(range(off, off + nl))
    off += nl
    cf_cols = off
    # bf16 const blob [128, 256]: cols 0:128 identity, cols 128:256 row0=ones
    cb_cols = 256

    nc = bacc.Bacc("TRN2", target_bir_lowering=False, debug=False, num_devices=NCORES)

    xa_d = nc.dram_tensor("xa", [c_in1, ng], BF16, kind="ExternalInput")
    xa1_d = nc.dram_tensor("xa1", [c_in1, n_loc], BF16, kind="ExternalInput")
    cstf_d = nc.dram_tensor("cstf", [P, cf_cols], F32, kind="ExternalInput")
    cstb_d = nc.dram_tensor("cstb", [P, cb_cols], BF16, kind="ExternalInput")
    srcw_d = nc.dram_tensor("srcw", [P, nbc * g_pad // 16], I16, kind="ExternalInput")
    ohd_d = nc.dram_tensor("ohd", [P, n_gt * 2 * P], BF16, kind="ExternalInput")
    w_d = [
        nc.dram_tensor(f"w{l + 1}", [dims[l][0] + 1, 3 * dims[l][1]], BF16, kind="ExternalInput")
        for l in range(nl)
    ]
    w3n_d = nc.dram_tensor("w3n", [dims[2][0], T2W], BF16, kind="ExternalInput")
    w3nb_d = nc.dram_tensor("w3nb", [1, T2W], BF16, kind="ExternalInput")
    out_d = nc.dram_tensor("out", [n_loc, c_out_last], F32, kind="ExternalOutput")

    with tile.TileContext(nc) as tc:
        with (
            tc.tile_pool(name="cst", bufs=1) as cpool,
            tc.tile_pool(name="persist", bufs=1) as ppool,
            tc.tile_pool(name="xch", bufs=2) as xpool,
            tc.tile_pool(name="xa", bufs=2) as xapool,
            tc.tile_pool(name="tw", bufs=2) as twpool,
            tc.tile_pool(name="work", bufs=3) as wpool,
            tc.tile_pool(name="msgp", bufs=6) as mpool,
            tc.tile_pool(name="gath", bufs=3) as gpool,
            tc.tile_pool(name="ohs", bufs=3) as opool,
            tc.tile_pool(name="ps", bufs=2, space="PSUM") as pspool,
            tc.tile_pool(name="hps", bufs=2, space="PSUM") as hpool,
            tc.tile_pool(name="acc", bufs=2, space="PSUM") as accpool,
            tc.tile_pool(name="dram", bufs=1, space="DRAM") as dpool,
        ):
            # ---------------- constants
            cstf = cpool.tile([P, cf_cols], F32)
            nc.sync.dma_start(cstf[:], cstf_d[:])
            cstb = cpool.tile([P, cb_cols], BF16)
            nc.sync.dma_start(cstb[:], cstb_d[:])
            srcw = cpool.tile([P, nbc * g_pad // 16], I16)
            nc.sync.dma_start(srcw[:], srcw_d[:])
            wfeat = []   # per layer: list of [<=128, 3co] bf16 SBUF tiles
            wbias = []   # per layer: [1, 3co] bias-row AP
            for l in range(nl):
                ci_l, co_l = dims[l]
                nk = (ci_l + P - 1) // P
                chunks = []
                for k in range(nk):
                    r0, r1 = k * P, min((k + 1) * P, ci_l)
                    w_t = cpool.tile([r1 - r0, 3 * co_l], BF16, tag=f"w{l}_{k}")
                    nc.sync.dma_start(w_t[:], w_d[l][:][r0:r1, :])
                    chunks.append(w_t)
                wb = cpool.tile([1, 3 * co_l], BF16, tag=f"w{l}_b")
                nc.sync.dma_start(wb[:], w_d[l][:][ci_l : ci_l + 1, :])
                wfeat.append(chunks)
                wbias.append(wb)
            w3n_t = cpool.tile([dims[2][0], T2W], BF16, tag="w3n")
            nc.sync.dma_start(w3n_t[:], w3n_d[:])
            w3nb_t = cpool.tile([1, T2W], BF16, tag="w3nb")
            nc.sync.dma_start(w3nb_t[:], w3nb_d[:])
            # [W' ; bx] (65, 256) for the final per-bin linear of layer 2
            w65_t = cpool.tile([dims[2][0] + 1, dims[2][1]], BF16, tag="w65")
            nc.sync.dma_start(w65_t[:], w_d[2][:][0 : dims[2][0] + 1, 0 : dims[2][1]])

            identb = cstb[:, 0:P]
            ones_row = cstb[0:1, P : P + P]

            # ---------------- persistent
            a_sb = ppool.tile([P, nbc * max(d[1] for d in dims)], BF16, tag="a_sb")
            xloc = ppool.tile([64, n_loc], BF16, tag="xloc")

            # ---------------- DRAM internals
            tables = [
                dpool.tile([ng, rw[l]], BF16, tag=f"table{l}", name=f"table{l}")
                for l in range(nl)
            ]
            # feature AllGather split in halves, overlapped with edge phase
            n_half = n_loc // 2
            ag_in = [
                [
                    dpool.tile([64, n_half], BF16, tag=f"agin{l}_{h}", name=f"agin{l}_{h}")
                    for h in range(2)
                ]
                for l in range(nl - 1)
            ]
            ag_out = [
                [
                    dpool.tile(
                        [NCORES * 64, n_half],
                        BF16,
                        tag=f"agout{l}_{h}",
                        name=f"agout{l}_{h}",
                        addr_space="Local" if SIM_MODE else "Shared",
                    )
                    for h in range(2)
                ]
                for l in range(nl - 1)
            ]

            def emit_ag(l, h):
                nc.sync.dma_start(
                    ag_in[l][h][:], xloc[:, h * n_half : (h + 1) * n_half]
                )
                if SIM_MODE:
                    for r in range(NCORES):
                        nc.sync.dma_start(
                            ag_out[l][h][:][r * 64 : (r + 1) * 64, :],
                            ag_in[l][h][:],
                        )
                else:
                    nc.gpsimd.collective_compute(
                        "AllGather",
                        ALU.bypass,
                        replica_groups=[list(range(NCORES))],
                        ins=[ag_in[l][h].opt()],
                        outs=[ag_out[l][h].opt()],
                    )

            for l in range(nl):
                ci, co = dims[l]
                table = tables[l]
                last = l == nl - 1

                # ======== A phase first: own bins (per-core lhsT inputs are
                # ready before the AllGather the node phase waits on, so PE
                # fills the collective latency with A-phase work)
                for t in range(nbc):
                    cols = slice(t * P, (t + 1) * P)
                    a_ps = hpool.tile([P, co], F32, space="PSUM", tag="psH")
                    if l == 0:
                        xa = xapool.tile([P, k1 * P], BF16, tag="xa")
                        xa3 = xa[:].rearrange("p (c n) -> p c n", c=k1)
                        nc.sync.dma_start(
                            xa3,
                            xa1_d[:, t * P : (t + 1) * P].rearrange(
                                "(c p) n -> p c n", p=P
                            ),
                        )
                        for k in range(k1):
                            nc.tensor.matmul(
                                out=a_ps[:],
                                lhsT=xa3[:, k, :],
                                rhs=wfeat[l][k][:, 2 * co : 3 * co],
                                start=(k == 0),
                                stop=False,
                            )
                    else:
                        nc.tensor.matmul(
                            out=a_ps[:],
                            lhsT=xloc[:, cols],
                            rhs=wfeat[l][0][:, 2 * co : 3 * co],
                            start=True,
                            stop=False,
                        )
                    nc.tensor.matmul(
                        out=a_ps[:],
                        lhsT=ones_row,
                        rhs=wbias[l][0:1, 2 * co : 3 * co],
                        start=False,
                        stop=True,
                    )
                    nc.vector.tensor_copy(
                        out=a_sb[:, t * co : (t + 1) * co], in_=a_ps[:]
                    )

                # ======== node phase: all nb bins, streamed in super-chunks.
                # For l>=1 the chunks are split by AllGather half (bins 0-14
                # of a core's chunk only need half 0), all half-0 sub-chunks
                # first, so 7/8 of the node phase overlaps the second
                # collective.
                if l == 0:
                    chunks = [
                        (r * nb1, min(nb1, cfg.nb - r * nb1)) for r in range(nch1)
                    ]
                else:
                    nbh = nbc // 2
                    chunks = [
                        (r * nbc + h * nbh, nbh)
                        for h in range(2)
                        for r in range(NCORES)
                    ]
                for b0, nbch in chunks:
                    w_ch = nbch * P
                    if l == 0:
                        xch = xpool.tile([P, k1 * nb1 * P], BF16, tag="xch")
                        xch3 = xch[:, 0 : k1 * w_ch].rearrange(
                            "p (c n) -> p c n", c=k1
                        )
                        nc.sync.dma_start(
                            xch3,
                            xa_d[:, b0 * P : b0 * P + w_ch].rearrange(
                                "(c p) n -> p c n", p=P
                            ),
                        )
                        kch = [xch3[:, k, :] for k in range(k1)]
                    else:
                        h = (b0 % nbc) // (nbc // 2)
                        r = b0 // nbc
                        xch = xpool.tile([64, n_half], BF16, tag="xch")
                        nc.sync.dma_start(
                            xch[:, 0:w_ch],
                            ag_out[l - 1][h][:][r * 64 : r * 64 + 64, :],
                        )
                        kch = [xch[:, 0:w_ch]]

                    # batched table write: TWB bins per dma_start
                    for t0 in range(0, nbch, TWB):
                        tn = min(TWB, nbch - t0)
                        twb = twpool.tile([P, TWB * rw[l]], BF16, tag="twb")
                        tw3 = twb[:, 0 : tn * rw[l]].rearrange(
                            "p (b d) -> p b d", d=rw[l]
                        )
                        for ti in range(tn):
                            t = t0 + ti
                            cols = slice(t * P, (t + 1) * P)
                            ps1 = pspool.tile([P, rw[l]], F32, space="PSUM", tag="psN")
                            if last:
                                nc.tensor.matmul(
                                    out=ps1[:], lhsT=kch[0][:, cols],
                                    rhs=w3n_t[:], start=True, stop=False,
                                )
                                nc.tensor.matmul(
                                    out=ps1[:], lhsT=ones_row,
                                    rhs=w3nb_t[:], start=False, stop=True,
                                )
                            else:
                                for k, kc in enumerate(kch):
                                    nc.tensor.matmul(
                                        out=ps1[:], lhsT=kc[:, cols],
                                        rhs=wfeat[l][k][:, 0 : 2 * co],
                                        start=(k == 0), stop=False,
                                    )
                                nc.tensor.matmul(
                                    out=ps1[:], lhsT=ones_row,
                                    rhs=wbias[l][0:1, 0 : 2 * co],
                                    start=False, stop=True,
                                )
                            if l == 1:
                                nc.vector.tensor_copy(out=tw3[:, ti, :], in_=ps1[:])
                            else:
                                nc.scalar.activation(
                                    out=tw3[:, ti, :], in_=ps1[:], func=AF.Copy
                                )
                        nc.sync.dma_start(
                            table[:][(b0 + t0) * P : (b0 + t0 + tn) * P, :].rearrange(
                                "(b p) d -> p b d", p=P
                            ),
                            tw3,
                        )

                # ======== edge phase: own bins
                w2rep = cstf[:, c_w2[l] : c_w2[l] + co]
                b2col = cstf[:, c_b2[l] : c_b2[l] + 1]
                mw = 65 if last else co        # msg width
                bs0 = P if last else co        # B-part column offset in row
                g3 = None
                oh3 = None
                o_ps = s_pre = s_sig = scr = None
                pend = []
                for gt in range(n_gt):
                    t, j = divmod(gt, nt_e)
                    if gt % GCH == 0:
                        hn = min(GCH, n_gt - gt)
                        gbuf = gpool.tile([P, GCH * rw[l]], BF16, tag="gbuf")
                        g3 = gbuf[:, 0 : hn * rw[l]].rearrange(
                            "p (j d) -> p j d", d=rw[l]
                        )
                        nc.gpsimd.dma_gather(
                            out_ap=g3,
                            in_ap=table[:],
                            idxs_ap=srcw[:, gt * 8 : (gt + hn) * 8],
                            num_idxs=hn * P,
                            num_idxs_reg=hn * P,
                            elem_size=rw[l],
                        )
                    if gt % OCH == 0:
                        on = min(OCH, n_gt - gt)
                        ohb = opool.tile([P, OCH * 2 * P], BF16, tag="ohb")
                        oh3 = ohb[:, 0 : on * 2 * P].rearrange(
                            "p (j d) -> p j d", d=2 * P
                        )
                        nc.sync.dma_start(
                            oh3,
                            ohd_d[:, gt * 2 * P : (gt + on) * 2 * P].rearrange(
                                "p (j d) -> p j d", d=2 * P
                            ),
                        )
                    gs = gt % GCH
                    os_ = gt % OCH
                    if j == 0:
                        if last:
                            o_ps = accpool.tile([65, P], F32, space="PSUM", tag="o_ps")
                        else:
                            o_ps = accpool.tile([64, P], F32, space="PSUM", tag="o_ps")
                        s_pre = wpool.tile([P, nt_e], F32, tag="s_pre")
                        s_sig = wpool.tile([P, nt_e], F32, tag="s_sig")
                        scr = wpool.tile([P, co], F32, tag="scr")
                    # h = A[dst] + B[src] in PSUM
                    h_ps = hpool.tile([P, co], F32, space="PSUM", tag="psH")
                    nc.tensor.matmul(
                        out=h_ps[:],
                        lhsT=oh3[:, os_, P : 2 * P],
                        rhs=a_sb[:, t * co : (t + 1) * co],
                        start=True,
                        stop=False,
                    )
                    nc.tensor.matmul(
                        out=h_ps[:],
                        lhsT=identb,
                        rhs=g3[:, gs, bs0 : bs0 + co],
                        start=False,
                        stop=True,
                    )
                    # s_pre[:, j] = sum(relu(h) * w2)
                    nc.vector.scalar_tensor_tensor(
                        out=scr[:],
                        in0=h_ps[:],
                        scalar=0.0,
                        in1=w2rep,
                        op0=ALU.max,
                        op1=ALU.mult,
                        accum_out=s_pre[:, j : j + 1],
                    )
                    pend.append((j, gs, os_, g3, oh3))
                    # sigmoid in SIGK batches, then flush pending msg+scatter
                    if (j + 1) % SIGK == 0 or j == nt_e - 1:
                        j0 = pend[0][0]
                        nc.scalar.activation(
                            out=s_sig[:, j0 : j + 1],
                            in_=s_pre[:, j0 : j + 1],
                            func=AF.Sigmoid,
                            bias=b2col,
                        )
                        for pj, pgs, pos, pg3, poh3 in pend:
                            msg = mpool.tile([P, mw], BF16, tag="msg")
                            if l == 1:
                                nc.gpsimd.tensor_tensor(
                                    out=msg[:],
                                    in0=pg3[:, pgs, 0:mw],
                                    in1=s_sig[:, pj : pj + 1].to_broadcast([P, mw]),
                                    op=ALU.mult,
                                )
                            else:
                                nc.scalar.activation(
                                    out=msg[:],
                                    in_=pg3[:, pgs, 0:mw],
                                    func=AF.Copy,
                                    scale=s_sig[:, pj : pj + 1],
                                )
                            nc.tensor.matmul(
                                out=o_ps[:],
                                lhsT=msg[:],
                                rhs=poh3[:, pos, 0:P],
                                start=(pj == 0),
                                stop=(pj == nt_e - 1),
                            )
                        pend = []
                    if j == nt_e - 1:
                        if not last:
                            nc.scalar.activation(
                                out=xloc[:, t * P : (t + 1) * P],
                                in_=o_ps[:],
                                func=AF.Relu,
                            )
                            if t == nbc // 2 - 1:
                                emit_ag(l, 0)
                            elif t == nbc - 1:
                                emit_ag(l, 1)
                        else:
                            m1sb = wpool.tile([65, P], BF16, tag="m1sb")
                            nc.scalar.activation(out=m1sb[:], in_=o_ps[:], func=AF.Copy)
                            ps_o = accpool.tile([P, co], F32, space="PSUM", tag="psO")
                            nc.tensor.matmul(
                                out=ps_o[:], lhsT=m1sb[:], rhs=w65_t[:],
                                start=True, stop=True,
                            )
                            ostg = wpool.tile([P, co], F32, tag="ostg")
                            nc.scalar.activation(out=ostg[:], in_=ps_o[:], func=AF.Copy)
                            nc.sync.dma_start(out_d[t * P : (t + 1) * P, :], ostg[:])

    nc.compile()
    return nc


# ---------------------------------------------------------------- driver

_PROG_CACHE = {}


def _build_in_maps(inputs, cfg):
    x = np.ascontiguousarray(np.asarray(inputs["x"], dtype=np.float32))
    ei = np.asarray(inputs["edge_index"]).astype(np.int64)
    src, dst = ei[0], ei[1]

    g_of, g_pad, per_core, xa_full = _host_prep(x, src, dst, cfg)
    fw = _fuse_weights(inputs, cfg)

    nl = len(cfg.dims)
    n_loc = cfg.nbc * P
    c_w2_w = sum(d[1] for d in cfg.dims)
    cf_cols = c_w2_w + nl
    cstf = np.zeros((P, cf_cols), np.float32)
    off = 0
    for l in range(nl):
        cstf[:, off : off + cfg.dims[l][1]] = fw[l]["w2"][None, :]
        off += cfg.dims[l][1]
    for l in range(nl):
        cstf[:, off] = fw[l]["b2"]
        off += 1
    cstb = np.zeros((P, 256), NPBF16)
    cstb[:, 0:128] = np.eye(128, dtype=np.float32).astype(NPBF16)
    cstb[0, 128:256] = 1.0

    in_maps = []
    for c in range(NCORES):
        srcw, ohd = per_core[c]
        in_maps.append(
            {
                "xa": xa_full,
                "xa1": np.ascontiguousarray(xa_full[:, c * n_loc : (c + 1) * n_loc]),
                "cstf": cstf,
                "cstb": cstb,
                "srcw": srcw,
                "ohd": ohd,
                "w3n": fw[-1]["w3n"],
                "w3nb": fw[-1]["w3nb"],
                **{f"w{l + 1}": fw[l]["wmat"] for l in range(nl)},
            }
        )
    return in_maps, g_of, g_pad


def _run(inputs, cfg, trace=False):
    in_maps, g_of, g_pad = _build_in_maps(inputs, cfg)

    key = (cfg.n_real, cfg.nbc, g_pad)
    if key not in _PROG_CACHE:
        _PROG_CACHE[key] = _build_program(cfg, g_pad)
    nc = _PROG_CACHE[key]

    res = run_bass_kernel_spmd(nc, in_maps, core_ids=list(range(NCORES)), trace=trace)

    n_loc = cfg.nbc * P
    full = np.empty((cfg.ng, cfg.dims[-1][1]), np.float32)
    for c in range(NCORES):
        full[c * n_loc : (c + 1) * n_loc] = res.results[c]["out"]
    out = full[g_of]
    return out, res


def kernel(**inputs) -> np.ndarray:
    out, _ = _run(inputs, CFG, trace=False)
    return out
